# revision 53
# baseline (speedup 1.0000x reference)
"""Trainium2 Bass kernel for the isotropic-gaussian differentiable renderer.

Math: for pixel p=(x,y) and gaussian g:
    w[g,p] = op_g * exp(-0.5*((x-ax_g)^2+(y-ay_g)^2)/var_g)
    img[p,c] = (sum_g w[g,p]*col_gc) / (sum_g w[g,p] + n_chunks*EPS)

The isotropic RBF is separable: w = op * exp(sx) * exp(sy) with
sx = s*(x-ax)^2, sy = s*(y-ay)^2 + ln(op), s = -0.5/var.  That turns the
268M-element exp into 2*N*128 exps plus matmuls:

  per 128-gaussian chunk:
    PE (f32r): arg[g, 0:128]=sx(g,x), arg[g,128:256]=sy(g,y) via a K=12
               matmul against fixed rows [u^2,u,1|v^2,v,1] duplicated for a
               hi/lo coefficient split (centered coords; the split keeps the
               catastrophically-cancelling quadratic exact in f32r)
    ACT      : exp(arg) -> fp16 written into fused per-chunk blocks
               [expx(128) | B(128) | colors(384)]; the y half lands as the
               den block B = op*expy directly (ln(op) is in the argument)
    DVE      : 3 tensor_scalar ops fill the color blocks col_c*B from the
               SAME rounded B, so fp16 weight rounding cancels in num/den
    PE (fp16): acc[x, (den|c)*128+y] += block[0:128]^T @ block[128:640]
               (fp32 PSUM accumulate; PE pre-warmed off memset tiles so
               f32r arg matmuls stream at 1 cyc/row)

Sharding: gaussians split 2048/core across 8 cores; every core accumulates
the full 128x128 image; host sums the 8 partials, divides num/den and
reshapes to the reference's [4,3,64,64] tile layout.
"""
import numpy as np

import concourse.bacc as bacc
import concourse.tile as tile
from concourse import mybir
from concourse.bass_utils import run_bass_kernel_spmd

# Problem constants (hardcoded per harness contract)
N_GAUSS = 16384
H = 128
W = 128
FX = 128.0
FY = 128.0
CX = 64.0
CY = 64.0
EPS = 1e-8
N_CORES = 8
G_PER_CORE = N_GAUSS // N_CORES      # 2048
CHUNK = 128                          # gaussians per matmul chunk
N_CHUNKS = G_PER_CORE // CHUNK       # 16
ARG_W = 256                          # per-chunk arg width: 128 x | 128 y
GROUP = 4                            # chunks per exp batch
N_GROUPS = N_CHUNKS // GROUP         # 4
OUT_W = 512                          # (c,y) free width of the accumulator

F32 = mybir.dt.float32
MM_DT = mybir.dt.float16             # main-accumulation matmul dtype.
# fp16 is safe here because of how A is factored: B = op*expy is rounded
# once and BOTH num and den consume the same rounded B (and the same
# rounded expx), so weight-rounding cancels in num/den; only the color
# weights carry an independent 2^-11 rounding, which averages out.
F32R = mybir.dt.float32r
KARG = 12                            # arg-matmul contraction: 6 coef rows x hi/lo
PACK = 4                             # arg matmuls packed per PE pass (row groups)
USE_PACK = False                     # tile_position matmuls crash TRN2 here; keep off


def build_program():
    """One SPMD Bass program; every core runs it on its gaussian slice."""
    nc = bacc.Bacc("TRN2", target_bir_lowering=False, debug=False,
                   num_devices=N_CORES)
    # packed: [128, 4*128]: coefpack[32k+r, grp*128+j] = coef row r of chunk
    # (grp*PACK+k), gaussian j — four chunks stacked at partition 0/32/64/96
    # so four K=6 arg matmuls run concurrently in separate PE row groups.
    # unpacked: [6, 2048] flat, one chunk per 128 columns.
    coef_shape = [128, N_GROUPS * CHUNK] if USE_PACK else [KARG, G_PER_CORE]
    coef = nc.dram_tensor("coef", coef_shape, F32, kind="ExternalInput")
    # the 6 fixed moving rows [u^2,u,1|0] / [0|v^2,v,1] (replicated at
    # partition bands 0/32/64/96 when packed).
    rhs_shape = [128, ARG_W] if USE_PACK else [KARG, ARG_W]
    rhsxy = nc.dram_tensor("rhsxy", rhs_shape, F32, kind="ExternalInput")
    # [128, 64]: opc[p, chunk*4+c] = (op*[r,g,b,1])[chunk*128+p, c]
    opc = nc.dram_tensor("opc", [128, N_CHUNKS * 4], F32, kind="ExternalInput")
    # partial accumulator: [x, c*128+y]
    out = nc.dram_tensor("out", [128, OUT_W], F32, kind="ExternalOutput")

    with tile.TileContext(nc) as tc:
        with tc.tile_pool(name="ins", bufs=1) as ins_pool, \
             tc.tile_pool(name="expp", bufs=1) as exp_pool, \
             tc.tile_pool(name="args", bufs=3, space="PSUM") as arg_pool, \
             tc.tile_pool(name="acc", bufs=1, space="PSUM") as acc_pool, \
             tc.tile_pool(name="warmp", bufs=1, space="PSUM") as warm_pool, \
             tc.tile_pool(name="outp", bufs=1) as out_pool:

            coef_t = ins_pool.tile(coef_shape, F32)
            rhs_t = ins_pool.tile(rhs_shape, F32)
            opc_t = ins_pool.tile([128, N_CHUNKS * 4], F32)
            # parallel triggers spread across engine queues; coef split by
            # group so group 0's arg matmuls start as soon as possible
            GW = CHUNK if USE_PACK else PACK * CHUNK  # coef cols per DMA slice
            NSPLIT = (coef_shape[1] + GW - 1) // GW
            nc.scalar.dma_start(out=rhs_t, in_=rhsxy[:, :])
            nc.sync.dma_start(out=coef_t[:, 0 * GW:1 * GW], in_=coef[:, 0 * GW:1 * GW])
            nc.scalar.dma_start(out=coef_t[:, 1 * GW:2 * GW], in_=coef[:, 1 * GW:2 * GW])
            nc.sync.dma_start(out=coef_t[:, 2 * GW:3 * GW], in_=coef[:, 2 * GW:3 * GW])
            nc.scalar.dma_start(out=coef_t[:, 3 * GW:4 * GW], in_=coef[:, 3 * GW:4 * GW])
            nc.gpsimd.dma_start(out=opc_t, in_=opc[:, :])

            # f32r operands must be produced by an on-chip rounding op; the
            # host pre-rounds to the f32r grid so these casts are exact.
            # Run the casts on ScalarE (idle until the first exp) to keep
            # the Vector engine free for the A-build.
            coef_r = ins_pool.tile(coef_shape, F32R)
            rhs_r = ins_pool.tile(rhs_shape, F32R)
            nc.vector.tensor_copy(rhs_r, rhs_t)
            casts_done = set()

            def ensure_cast(sl):
                if sl in casts_done:
                    return
                casts_done.add(sl)
                nc.vector.tensor_copy(coef_r[:, sl * GW:(sl + 1) * GW],
                                      coef_t[:, sl * GW:(sl + 1) * GW])

            # fused per-chunk block [expx(128) | B(128) | colors(384)]:
            # the exp writes [x|y] at block start (y IS B = op*expy), the
            # DVE writes the color blocks, and the main matmul reads
            # lhsT = block[0:128], rhs = block[128:640] with no extra copy.
            BLK = 640
            t3 = exp_pool.tile([128, N_CHUNKS, BLK], MM_DT)
            acc = acc_pool.tile([128, OUT_W], F32)

            # PE warmup off memset tiles (ready ~6us, before any input DMA
            # lands): ~3us of dummy matmuls flips the HAM clock gate to 8/8
            # so the real arg matmuls run at 2.4 GHz, in otherwise-dead time.
            wsrc = ins_pool.tile([128, ARG_W], mybir.dt.bfloat16)
            nc.gpsimd.memset(wsrc, 0.0)
            wdst = warm_pool.tile([128, ARG_W], F32)
            for _ in range(9):
                nc.tensor.matmul(wdst[:, :], wsrc[:, :CHUNK], wsrc[:, :],
                                 start=True, stop=True)

            # narrow leading groups tighten the pipeline front: chunk 0's
            # A-build waits on a 1-chunk exp; later coef casts are emitted
            # just-in-time so they don't block the first A-builds on DVE.
            group_plan = [(0, 1), (1, 1), (2, 2), (4, 4), (8, 4), (12, 4)]
            for g0c, width in group_plan:
                for k in range(width):
                    ensure_cast((g0c + k) * CHUNK // GW)
                args = arg_pool.tile([128, width * ARG_W], F32, tag="args")
                for k in range(width):
                    chunk = g0c + k
                    nc.tensor.matmul(
                        args[:, k * ARG_W:(k + 1) * ARG_W],
                        coef_r[:, chunk * CHUNK:(chunk + 1) * CHUNK],
                        rhs_r[:, :],
                        start=True, stop=True,
                    )
                nc.scalar.activation(
                    out=t3[:, g0c:g0c + width, 0:ARG_W],
                    in_=args[:, :width * ARG_W],
                    func=mybir.ActivationFunctionType.Exp,
                )

            for chunk in range(N_CHUNKS):
                # y half of the exp is B = op*expy (ln(op) in the arg);
                # color blocks multiply the SAME rounded B so num/den
                # rounding cancels.  Accumulator column order: [den|r|g|b].
                for c in range(3):
                    nc.vector.tensor_scalar_mul(
                        out=t3[:, chunk, 256 + c * 128:256 + (c + 1) * 128],
                        in0=t3[:, chunk, 128:256],
                        scalar1=opc_t[:, chunk * 4 + c:chunk * 4 + c + 1],
                    )
                nc.tensor.matmul(
                    acc[:, :],
                    t3[:, chunk, 0:128],
                    t3[:, chunk, 128:BLK],
                    start=(chunk == 0), stop=(chunk == N_CHUNKS - 1),
                )

            out_t = out_pool.tile([128, OUT_W], F32)
            nc.scalar.copy(out=out_t[:, :256], in_=acc[:, :256])
            nc.scalar.dma_start(out=out[:, :256], in_=out_t[:, :256])
            nc.scalar.copy(out=out_t[:, 256:], in_=acc[:, 256:])
            nc.sync.dma_start(out=out[:, 256:], in_=out_t[:, 256:])

    nc.compile()
    return nc


_PROGRAM = None


def _get_program():
    global _PROGRAM
    if _PROGRAM is None:
        _PROGRAM = build_program()
    return _PROGRAM


def _quat2mat(q):
    q = q / np.linalg.norm(q)
    w, x, y, z = q
    return np.array([
        [1 - 2 * (y * y + z * z), 2 * (x * y - z * w), 2 * (x * z + y * w)],
        [2 * (x * y + z * w), 1 - 2 * (x * x + z * z), 2 * (y * z - x * w)],
        [2 * (x * z - y * w), 2 * (y * z + x * w), 1 - 2 * (x * x + y * y)],
    ])


def kernel(positions, colors, opacities, scales, qvec, tvec, tile_hw,
           chunk_gauss, _trace=False):
    positions = np.asarray(positions, dtype=np.float32)
    colors = np.asarray(colors, dtype=np.float32)
    opacities = np.asarray(opacities, dtype=np.float32)
    scales = np.asarray(scales, dtype=np.float32)
    qvec = np.asarray(qvec, dtype=np.float32)
    tvec = np.asarray(tvec, dtype=np.float32)
    tile_hw = int(tile_hw)
    chunk_gauss = int(chunk_gauss)
    n = positions.shape[0]
    assert n == N_GAUSS, f"expected {N_GAUSS} gaussians, got {n}"

    # ---- O(N) per-gaussian prep in float64 (rounds to the same f32 values
    # the reference computes, to well within the exp's own error budget) ----
    R = _quat2mat(qvec.astype(np.float64))
    cam = positions.astype(np.float64) @ R.T + tvec.astype(np.float64)
    ax = cam[:, 0] / cam[:, 2] * FX + CX          # [N] screen x center
    ay = cam[:, 1] / cam[:, 2] * FY + CY          # [N] screen y center
    var = scales[:, 0].astype(np.float64) ** 2
    s = -0.5 / var                                # [N] negative inv 2*var

    # centered coords keep the quadratic-expansion terms small (|u|<=64)
    dx = ax - CX
    dy = ay - CY

    def f32r_round(x):
        """Round to the f32r grid (low 12 mantissa bits of fp32 cleared)."""
        v32 = np.asarray(x, dtype=np.float32).view(np.uint32)
        return ((v32 + 0x800) & np.uint32(0xFFFFF000)).view(np.float32)

    def hilo(x):
        """Split x into f32r-representable hi+lo with hi+lo ~= x to ~2^-24."""
        hi = f32r_round(x).astype(np.float64)
        lo = f32r_round(np.asarray(x, dtype=np.float64) - hi)
        return hi.astype(np.float32), lo.astype(np.float32)

    # K=12 stationary rows per gaussian (hi/lo pairs), for
    #   arg_x = s*u^2 + (-2 s dx)*u + s*dx^2     (u = x - 64)
    #   arg_y = s*v^2 + (-2 s dy)*v + s*dy^2     (v = y - 64)
    # u^2 <= 4096 is exact in f32r (12-bit significand), so hi-row products
    # are exact in the PE and lo rows mop up the residue: the f32r arg
    # matmul matches fp32 to ~1e-6 despite the quadratic cancellation.
    # +ln(op) on the y-constant row makes exp(arg_y) = op*exp_y directly
    op64 = opacities[:, 0].astype(np.float64)
    rows6 = [s, -2.0 * s * dx, s * dx * dx,
             s, -2.0 * s * dy, s * dy * dy + np.log(op64)]
    coef_rows = []
    for r in rows6:
        hi, lo = hilo(r)
        coef_rows.extend([hi, lo])
    coef_full = np.stack(coef_rows).astype(np.float32)   # [12, N]

    u = np.arange(W, dtype=np.float64) - CX
    v = np.arange(H, dtype=np.float64) - CY
    zeros = np.zeros(128)
    ones = np.ones(128)
    rhs_rows = []
    for base in (u * u, u, ones):
        row = np.concatenate([base, zeros]).astype(np.float32)
        rhs_rows.extend([row, row])   # hi and lo coef rows share the base
    for base in (v * v, v, ones):
        row = np.concatenate([zeros, base]).astype(np.float32)
        rhs_rows.extend([row, row])
    rhs6 = np.stack(rhs_rows)                             # [12, 256]
    if USE_PACK:
        # replicate at partition bands 0/32/64/96 for the row-group packing
        rhsxy = np.zeros((128, ARG_W), dtype=np.float32)
        for k in range(PACK):
            rhsxy[32 * k:32 * k + KARG] = rhs6
    else:
        rhsxy = rhs6

    # [N, 4] = [r, g, b, 1]: op is folded into the exp's y-argument
    opc_full = np.concatenate(
        [colors.astype(np.float64), np.ones((n, 1))], axis=1
    ).astype(np.float32)

    # ---- shard gaussians across the 8 cores ----
    in_maps = []
    for core in range(N_CORES):
        g0 = core * G_PER_CORE
        g1 = g0 + G_PER_CORE
        opc_c = opc_full[g0:g1].reshape(N_CHUNKS, CHUNK, 4)
        opc_c = np.ascontiguousarray(
            opc_c.transpose(1, 0, 2).reshape(CHUNK, N_CHUNKS * 4))
        if USE_PACK:
            # coefpack[32k+r, grp*128+j] = coef row r of chunk grp*PACK+k
            cc = coef_full[:, g0:g1].reshape(KARG, N_GROUPS, PACK, CHUNK)
            coefpack = np.zeros((128, N_GROUPS * CHUNK), dtype=np.float32)
            for k in range(PACK):
                coefpack[32 * k:32 * k + KARG] = (
                    cc[:, :, k, :].reshape(KARG, N_GROUPS * CHUNK))
        else:
            coefpack = np.ascontiguousarray(coef_full[:, g0:g1])
        in_maps.append({
            "coef": coefpack,
            "rhsxy": rhsxy,
            "opc": opc_c,
        })

    nc = _get_program()
    res = run_bass_kernel_spmd(nc, in_maps, list(range(N_CORES)),
                               trace=_trace)

    # ---- host reduction: sum per-core partials, divide, reshape ----
    acc = np.zeros((128, 4, 128), dtype=np.float64)   # [x, (den|r|g|b), y]
    for core in range(N_CORES):
        acc += res.results[core]["out"].reshape(128, 4, 128)

    num = acc[:, 1:4, :]                          # [x, c, y]
    n_chunks_ref = n // chunk_gauss
    den = acc[:, 0, :] + n_chunks_ref * EPS       # [x, y]
    img = num / den[:, None, :]                   # [x, c, y]
    img = img.transpose(2, 0, 1).reshape(H * W, 3)  # [p=(y,x), c]

    step = tile_hw * tile_hw
    t = (H * W) // step
    out = img.reshape(t, step, 3).transpose(0, 2, 1).reshape(
        t, 3, tile_hw, tile_hw)
    result = out.astype(np.float32)
    if _trace:
        return result, res
    return result


# revision 54
# speedup vs baseline: 1.0414x; 1.0414x over previous
"""Trainium2 Bass kernel for the isotropic-gaussian differentiable renderer.

Math: for pixel p=(x,y) and gaussian g:
    w[g,p] = op_g * exp(-0.5*((x-ax_g)^2+(y-ay_g)^2)/var_g)
    img[p,c] = (sum_g w[g,p]*col_gc) / (sum_g w[g,p] + n_chunks*EPS)

The isotropic RBF is separable: w = op * exp(sx) * exp(sy) with
sx = s*(x-ax)^2, sy = s*(y-ay)^2 + ln(op), s = -0.5/var.  That turns the
268M-element exp into 2*N*128 exps plus matmuls:

  per 128-gaussian chunk:
    PE (f32r): arg[g, 0:128]=sx(g,x), arg[g,128:256]=sy(g,y) via a K=12
               matmul against fixed rows [u^2,u,1|v^2,v,1] duplicated for a
               hi/lo coefficient split (centered coords; the split keeps the
               catastrophically-cancelling quadratic exact in f32r)
    ACT      : exp(arg) -> fp16 written into fused per-chunk blocks
               [expx(128) | B(128) | colors(384)]; the y half lands as the
               den block B = op*expy directly (ln(op) is in the argument)
    DVE      : 3 tensor_scalar ops fill the color blocks col_c*B from the
               SAME rounded B, so fp16 weight rounding cancels in num/den
    PE (fp16): acc[x, (den|c)*128+y] += block[0:128]^T @ block[128:640]
               (fp32 PSUM accumulate; PE pre-warmed off memset tiles so
               f32r arg matmuls stream at 1 cyc/row)

Sharding: gaussians split 2048/core across 8 cores; every core accumulates
the full 128x128 image; host sums the 8 partials, divides num/den and
reshapes to the reference's [4,3,64,64] tile layout.
"""
import numpy as np

import concourse.bacc as bacc
import concourse.tile as tile
from concourse import mybir
from concourse.bass_utils import run_bass_kernel_spmd

# Problem constants (hardcoded per harness contract)
N_GAUSS = 16384
H = 128
W = 128
FX = 128.0
FY = 128.0
CX = 64.0
CY = 64.0
EPS = 1e-8
N_CORES = 8
G_PER_CORE = N_GAUSS // N_CORES      # 2048
CHUNK = 128                          # gaussians per matmul chunk
N_CHUNKS = G_PER_CORE // CHUNK       # 16
ARG_W = 256                          # per-chunk arg width: 128 x | 128 y
GROUP = 4                            # chunks per exp batch
N_GROUPS = N_CHUNKS // GROUP         # 4
OUT_W = 512                          # (c,y) free width of the accumulator

F32 = mybir.dt.float32
MM_DT = mybir.dt.float16             # main-accumulation matmul dtype.
# fp16 is safe here because of how A is factored: B = op*expy is rounded
# once and BOTH num and den consume the same rounded B (and the same
# rounded expx), so weight-rounding cancels in num/den; only the color
# weights carry an independent 2^-11 rounding, which averages out.
F32R = mybir.dt.float32r
KARG = 12                            # arg-matmul contraction: 6 coef rows x hi/lo
PACK = 4                             # arg matmuls packed per PE pass (row groups)
USE_PACK = False                     # tile_position matmuls crash TRN2 here; keep off


def build_program():
    """One SPMD Bass program; every core runs it on its gaussian slice."""
    nc = bacc.Bacc("TRN2", target_bir_lowering=False, debug=False,
                   num_devices=N_CORES)
    # packed: [128, 4*128]: coefpack[32k+r, grp*128+j] = coef row r of chunk
    # (grp*PACK+k), gaussian j — four chunks stacked at partition 0/32/64/96
    # so four K=6 arg matmuls run concurrently in separate PE row groups.
    # unpacked: [6, 2048] flat, one chunk per 128 columns.
    coef_shape = [128, N_GROUPS * CHUNK] if USE_PACK else [KARG, G_PER_CORE]
    coef = nc.dram_tensor("coef", coef_shape, F32, kind="ExternalInput")
    # the 6 fixed moving rows [u^2,u,1|0] / [0|v^2,v,1] (replicated at
    # partition bands 0/32/64/96 when packed).
    rhs_shape = [128, ARG_W] if USE_PACK else [KARG, ARG_W]
    rhsxy = nc.dram_tensor("rhsxy", rhs_shape, F32, kind="ExternalInput")
    # [128, 64]: opc[p, chunk*4+c] = (op*[r,g,b,1])[chunk*128+p, c]
    opc = nc.dram_tensor("opc", [128, N_CHUNKS * 4], F32, kind="ExternalInput")
    # partial accumulator: [x, c*128+y]
    out = nc.dram_tensor("out", [128, OUT_W], F32, kind="ExternalOutput")

    with tile.TileContext(nc) as tc:
        with tc.tile_pool(name="ins", bufs=1) as ins_pool, \
             tc.tile_pool(name="expp", bufs=1) as exp_pool, \
             tc.tile_pool(name="args", bufs=3, space="PSUM") as arg_pool, \
             tc.tile_pool(name="acc", bufs=1, space="PSUM") as acc_pool, \
             tc.tile_pool(name="warmp", bufs=1, space="PSUM") as warm_pool, \
             tc.tile_pool(name="outp", bufs=1) as out_pool:

            coef_t = ins_pool.tile(coef_shape, F32)
            rhs_t = ins_pool.tile(rhs_shape, F32)
            opc_t = ins_pool.tile([128, N_CHUNKS * 4], F32)
            # parallel triggers spread across engine queues; coef split by
            # group so group 0's arg matmuls start as soon as possible
            GW = CHUNK if USE_PACK else PACK * CHUNK  # coef cols per DMA slice
            NSPLIT = (coef_shape[1] + GW - 1) // GW
            nc.scalar.dma_start(out=rhs_t, in_=rhsxy[:, :])
            nc.sync.dma_start(out=coef_t[:, 0 * GW:1 * GW], in_=coef[:, 0 * GW:1 * GW])
            nc.scalar.dma_start(out=coef_t[:, 1 * GW:2 * GW], in_=coef[:, 1 * GW:2 * GW])
            nc.sync.dma_start(out=coef_t[:, 2 * GW:3 * GW], in_=coef[:, 2 * GW:3 * GW])
            nc.scalar.dma_start(out=coef_t[:, 3 * GW:4 * GW], in_=coef[:, 3 * GW:4 * GW])
            nc.gpsimd.dma_start(out=opc_t, in_=opc[:, :])

            # f32r operands must be produced by an on-chip rounding op; the
            # host pre-rounds to the f32r grid so these casts are exact.
            # Run the casts on ScalarE (idle until the first exp) to keep
            # the Vector engine free for the A-build.
            coef_r = ins_pool.tile(coef_shape, F32R)
            rhs_r = ins_pool.tile(rhs_shape, F32R)
            nc.vector.tensor_copy(rhs_r, rhs_t)
            for g in range(NSPLIT):
                nc.vector.tensor_copy(coef_r[:, g * GW:(g + 1) * GW],
                                      coef_t[:, g * GW:(g + 1) * GW])

            # fused per-chunk block [expx(128) | B(128) | colors(384)]:
            # the exp writes [x|y] at block start (y IS B = op*expy), the
            # DVE writes the color blocks, and the main matmul reads
            # lhsT = block[0:128], rhs = block[128:640] with no extra copy.
            BLK = 640
            t3 = exp_pool.tile([128, N_CHUNKS, BLK], MM_DT)
            acc = acc_pool.tile([128, OUT_W], F32)

            # PE warmup off memset tiles (ready ~6us, before any input DMA
            # lands): ~3us of dummy matmuls flips the HAM clock gate to 8/8
            # so the real arg matmuls run at 2.4 GHz, in otherwise-dead time.
            wsrc = ins_pool.tile([128, ARG_W], mybir.dt.bfloat16)
            nc.gpsimd.memset(wsrc, 0.0)
            wdst = warm_pool.tile([128, ARG_W], F32)
            for _ in range(9):
                nc.tensor.matmul(wdst[:, :], wsrc[:, :CHUNK], wsrc[:, :],
                                 start=True, stop=True)

            # narrow leading groups tighten the pipeline front: chunk 0's
            # A-build waits on a 1-chunk exp instead of a 4-chunk batch
            group_plan = [(0, 1), (1, 1), (2, 2), (4, 4), (8, 4), (12, 4)]
            for g0c, width in group_plan:
                args = arg_pool.tile([128, width * ARG_W], F32, tag="args")
                for k in range(width):
                    chunk = g0c + k
                    nc.tensor.matmul(
                        args[:, k * ARG_W:(k + 1) * ARG_W],
                        coef_r[:, chunk * CHUNK:(chunk + 1) * CHUNK],
                        rhs_r[:, :],
                        start=True, stop=True,
                    )
                nc.scalar.activation(
                    out=t3[:, g0c:g0c + width, 0:ARG_W],
                    in_=args[:, :width * ARG_W],
                    func=mybir.ActivationFunctionType.Exp,
                )

            for chunk in range(N_CHUNKS):
                # y half of the exp is B = op*expy (ln(op) in the arg);
                # color blocks multiply the SAME rounded B so num/den
                # rounding cancels.  Accumulator column order: [den|r|g|b].
                for c in range(3):
                    nc.vector.tensor_scalar_mul(
                        out=t3[:, chunk, 256 + c * 128:256 + (c + 1) * 128],
                        in0=t3[:, chunk, 128:256],
                        scalar1=opc_t[:, chunk * 4 + c:chunk * 4 + c + 1],
                    )
                nc.tensor.matmul(
                    acc[:, :],
                    t3[:, chunk, 0:128],
                    t3[:, chunk, 128:BLK],
                    start=(chunk == 0), stop=(chunk == N_CHUNKS - 1),
                )

            out_t = out_pool.tile([128, OUT_W], F32)
            nc.scalar.copy(out=out_t[:, :256], in_=acc[:, :256])
            nc.scalar.dma_start(out=out[:, :256], in_=out_t[:, :256])
            nc.scalar.copy(out=out_t[:, 256:], in_=acc[:, 256:])
            nc.sync.dma_start(out=out[:, 256:], in_=out_t[:, 256:])

    nc.compile()
    return nc


_PROGRAM = None


def _get_program():
    global _PROGRAM
    if _PROGRAM is None:
        _PROGRAM = build_program()
    return _PROGRAM


def _quat2mat(q):
    q = q / np.linalg.norm(q)
    w, x, y, z = q
    return np.array([
        [1 - 2 * (y * y + z * z), 2 * (x * y - z * w), 2 * (x * z + y * w)],
        [2 * (x * y + z * w), 1 - 2 * (x * x + z * z), 2 * (y * z - x * w)],
        [2 * (x * z - y * w), 2 * (y * z + x * w), 1 - 2 * (x * x + y * y)],
    ])


def kernel(positions, colors, opacities, scales, qvec, tvec, tile_hw,
           chunk_gauss, _trace=False):
    positions = np.asarray(positions, dtype=np.float32)
    colors = np.asarray(colors, dtype=np.float32)
    opacities = np.asarray(opacities, dtype=np.float32)
    scales = np.asarray(scales, dtype=np.float32)
    qvec = np.asarray(qvec, dtype=np.float32)
    tvec = np.asarray(tvec, dtype=np.float32)
    tile_hw = int(tile_hw)
    chunk_gauss = int(chunk_gauss)
    n = positions.shape[0]
    assert n == N_GAUSS, f"expected {N_GAUSS} gaussians, got {n}"

    # ---- O(N) per-gaussian prep in float64 (rounds to the same f32 values
    # the reference computes, to well within the exp's own error budget) ----
    R = _quat2mat(qvec.astype(np.float64))
    cam = positions.astype(np.float64) @ R.T + tvec.astype(np.float64)
    ax = cam[:, 0] / cam[:, 2] * FX + CX          # [N] screen x center
    ay = cam[:, 1] / cam[:, 2] * FY + CY          # [N] screen y center
    var = scales[:, 0].astype(np.float64) ** 2
    s = -0.5 / var                                # [N] negative inv 2*var

    # centered coords keep the quadratic-expansion terms small (|u|<=64)
    dx = ax - CX
    dy = ay - CY

    def f32r_round(x):
        """Round to the f32r grid (low 12 mantissa bits of fp32 cleared)."""
        v32 = np.asarray(x, dtype=np.float32).view(np.uint32)
        return ((v32 + 0x800) & np.uint32(0xFFFFF000)).view(np.float32)

    def hilo(x):
        """Split x into f32r-representable hi+lo with hi+lo ~= x to ~2^-24."""
        hi = f32r_round(x).astype(np.float64)
        lo = f32r_round(np.asarray(x, dtype=np.float64) - hi)
        return hi.astype(np.float32), lo.astype(np.float32)

    # K=12 stationary rows per gaussian (hi/lo pairs), for
    #   arg_x = s*u^2 + (-2 s dx)*u + s*dx^2     (u = x - 64)
    #   arg_y = s*v^2 + (-2 s dy)*v + s*dy^2     (v = y - 64)
    # u^2 <= 4096 is exact in f32r (12-bit significand), so hi-row products
    # are exact in the PE and lo rows mop up the residue: the f32r arg
    # matmul matches fp32 to ~1e-6 despite the quadratic cancellation.
    # +ln(op) on the y-constant row makes exp(arg_y) = op*exp_y directly
    op64 = opacities[:, 0].astype(np.float64)
    rows6 = [s, -2.0 * s * dx, s * dx * dx,
             s, -2.0 * s * dy, s * dy * dy + np.log(op64)]
    coef_rows = []
    for r in rows6:
        hi, lo = hilo(r)
        coef_rows.extend([hi, lo])
    coef_full = np.stack(coef_rows).astype(np.float32)   # [12, N]

    u = np.arange(W, dtype=np.float64) - CX
    v = np.arange(H, dtype=np.float64) - CY
    zeros = np.zeros(128)
    ones = np.ones(128)
    rhs_rows = []
    for base in (u * u, u, ones):
        row = np.concatenate([base, zeros]).astype(np.float32)
        rhs_rows.extend([row, row])   # hi and lo coef rows share the base
    for base in (v * v, v, ones):
        row = np.concatenate([zeros, base]).astype(np.float32)
        rhs_rows.extend([row, row])
    rhs6 = np.stack(rhs_rows)                             # [12, 256]
    if USE_PACK:
        # replicate at partition bands 0/32/64/96 for the row-group packing
        rhsxy = np.zeros((128, ARG_W), dtype=np.float32)
        for k in range(PACK):
            rhsxy[32 * k:32 * k + KARG] = rhs6
    else:
        rhsxy = rhs6

    # [N, 4] = [r, g, b, 1]: op is folded into the exp's y-argument
    opc_full = np.concatenate(
        [colors.astype(np.float64), np.ones((n, 1))], axis=1
    ).astype(np.float32)

    # ---- shard gaussians across the 8 cores ----
    in_maps = []
    for core in range(N_CORES):
        g0 = core * G_PER_CORE
        g1 = g0 + G_PER_CORE
        opc_c = opc_full[g0:g1].reshape(N_CHUNKS, CHUNK, 4)
        opc_c = np.ascontiguousarray(
            opc_c.transpose(1, 0, 2).reshape(CHUNK, N_CHUNKS * 4))
        if USE_PACK:
            # coefpack[32k+r, grp*128+j] = coef row r of chunk grp*PACK+k
            cc = coef_full[:, g0:g1].reshape(KARG, N_GROUPS, PACK, CHUNK)
            coefpack = np.zeros((128, N_GROUPS * CHUNK), dtype=np.float32)
            for k in range(PACK):
                coefpack[32 * k:32 * k + KARG] = (
                    cc[:, :, k, :].reshape(KARG, N_GROUPS * CHUNK))
        else:
            coefpack = np.ascontiguousarray(coef_full[:, g0:g1])
        in_maps.append({
            "coef": coefpack,
            "rhsxy": rhsxy,
            "opc": opc_c,
        })

    nc = _get_program()
    res = run_bass_kernel_spmd(nc, in_maps, list(range(N_CORES)),
                               trace=_trace)

    # ---- host reduction: sum per-core partials, divide, reshape ----
    acc = np.zeros((128, 4, 128), dtype=np.float64)   # [x, (den|r|g|b), y]
    for core in range(N_CORES):
        acc += res.results[core]["out"].reshape(128, 4, 128)

    num = acc[:, 1:4, :]                          # [x, c, y]
    n_chunks_ref = n // chunk_gauss
    den = acc[:, 0, :] + n_chunks_ref * EPS       # [x, y]
    img = num / den[:, None, :]                   # [x, c, y]
    img = img.transpose(2, 0, 1).reshape(H * W, 3)  # [p=(y,x), c]

    step = tile_hw * tile_hw
    t = (H * W) // step
    out = img.reshape(t, step, 3).transpose(0, 2, 1).reshape(
        t, 3, tile_hw, tile_hw)
    result = out.astype(np.float32)
    if _trace:
        return result, res
    return result
